# revision 20
# baseline (speedup 1.0000x reference)
# GAT (2-layer, single-head, PyG-style) on 8 Trainium2 NeuronCores.
#
# Strategy (dst-sharded, graph/data parallel):
#   - Nodes are ranked by in-degree (incl. self loop) and dealt round-robin
#     across the 8 cores, so window w of every core covers the same global
#     degree band ("degree-sorted ragged windows" -> minimal padding).
#   - Each core owns NPC node slots (windows of 128 dst slots). The incoming
#     edges of a window are laid out [128 dst, CT columns], columns grouped
#     into 4 bands by source-table range (dma_gather uses int16 indices, so
#     the gather table is split into 4 range tables of <=32k rows).
#   - Three bass kernels composed in ONE jitted XLA program via shard_map
#     over the 8 cores, with jax-level all_gathers between them (the halo
#     exchange of the node tables):
#       K1: per-node transform x@W1 -> table shard T1=[h|alpha_src|pad] (256B
#           rows, the dma_gather granularity) + resident dst-attention ad1.
#       K2: layer-1 edge aggregation (gather + softmax + weighted sum) +
#           h@W2 -> T2 shard + ad2.
#       K3: layer-2 edge aggregation + final h@Wf+bf -> output shard.
#   - Per window: one dma_gather per range band; padded slots gather a dummy
#     row (alpha_src=-1e30 => softmax weight exactly 0).
#   - w = exp(leaky_relu(as+ad)) on ACT (ad is a per-partition scalar since
#     dst sits on partitions; denominator via Exp accum_out; leaky relu
#     decomposed as 0.2e + 0.8 relu(e)). Weighted feature sum on DVE.
#     Softmax max-subtraction is skipped (exponents are O(1); the result is
#     mathematically identical).
#
# kernel(**inputs) takes FULL inputs, returns the FULL [N, 32] output.

import functools
import math

import numpy as np

P = 128
FIN = 128
FH = 32
CORES = 8
NRANGE = 4
MAXCOLS = 31          # max band columns per dma_gather call (desc-ring limit)
NEG_BIG = -1.0e30
TW = 64               # table row width (f32) = 256B
BW = 4                # windows per store batch


def _make_meta(edge_index, n_nodes):
    src = np.asarray(edge_index[0]).astype(np.int64)
    dst = np.asarray(edge_index[1]).astype(np.int64)
    loops = np.arange(n_nodes, dtype=np.int64)
    src_all = np.concatenate([src, loops])
    dst_all = np.concatenate([dst, loops])

    deg = np.bincount(dst_all, minlength=n_nodes)
    order = np.argsort(-deg, kind="stable")
    rank = np.empty(n_nodes, dtype=np.int64)
    rank[order] = np.arange(n_nodes)

    assert n_nodes % CORES == 0
    n_real_pc = n_nodes // CORES
    W = math.ceil((n_real_pc + 1) / P)
    npc = W * P
    assert n_real_pc < npc
    assert npc % NRANGE == 0
    QS = npc // NRANGE                  # quarter-shard rows per core
    RT = CORES * QS                     # rows per range table (+1 dummy)
    DUMMY = RT                          # dummy row index in every range table
    assert RT + 1 <= 32768, f"range table too big for int16: {RT}"

    core_of_node = rank % CORES
    slot_of_node = rank // CORES

    e_dc = core_of_node[dst_all]
    e_ds = slot_of_node[dst_all]
    e_sq = slot_of_node[src_all] // QS
    e_si = (core_of_node[src_all] * QS + slot_of_node[src_all] % QS).astype(
        np.int64)

    key = (e_dc * npc + e_ds) * NRANGE + e_sq
    sortp = np.argsort(key, kind="stable")
    key_s = key[sortp]
    e_si_s = e_si[sortp]

    counts = np.bincount(key_s, minlength=CORES * npc * NRANGE)
    cnt4 = counts.reshape(CORES, W, P, NRANGE)
    Dq = cnt4.max(axis=(0, 2))          # [W, NRANGE]
    Dq = np.maximum(Dq, 1)

    band_off = np.zeros((W, NRANGE), dtype=np.int64)
    band_off[:, 1:] = np.cumsum(Dq, axis=1)[:, :-1]
    CT = Dq.sum(axis=1)
    win_off = np.concatenate([[0], np.cumsum(CT)])
    TOTC = int(win_off[-1])

    seg_start = np.concatenate([[0], np.cumsum(counts)])
    k = np.arange(len(key_s)) - seg_start[key_s]
    cwpq = key_s
    q = cwpq % NRANGE
    p = (cwpq // NRANGE) % P
    w = (cwpq // (NRANGE * P)) % W
    c = cwpq // (NRANGE * P * W)
    col = win_off[w] + band_off[w, q] + k
    flatpos = col * P + p

    slots = np.full((CORES, TOTC * P), DUMMY, dtype=np.int16)
    slots[c, flatpos] = e_si_s.astype(np.int16)

    calls = []
    idx_blocks = {}
    for cc in range(CORES):
        blocks = []
        off16 = 0
        for ww in range(W):
            for qq in range(NRANGE):
                lo = int(band_off[ww, qq])
                width = int(Dq[ww, qq])
                while width > 0:
                    ncols = min(width, MAXCOLS)
                    a = (win_off[ww] + lo) * P
                    b = a + ncols * P
                    blk = slots[cc, a:b].reshape(ncols * 8, 16).T
                    blocks.append(blk)
                    if cc == 0:
                        calls.append((ww, qq, ncols, off16))
                    off16 += ncols * 8
                    lo += ncols
                    width -= ncols
        idx_blocks[cc] = np.ascontiguousarray(
            np.tile(np.concatenate(blocks, axis=1), (CORES, 1)),
            dtype=np.int16)

    return dict(
        order=order, rank=rank, npc=npc, n_real_pc=n_real_pc, W=W, QS=QS,
        RT=RT, DUMMY=int(DUMMY), CT=[int(x) for x in CT],
        calls=calls, idx=idx_blocks, IDXW=int(idx_blocks[0].shape[1]),
    )


def _build_fns(meta):
    import concourse.mybir as mybir
    import concourse.tile as tile
    from concourse.bass2jax import bass_jit
    from concourse.masks import make_identity

    dt = mybir.dt
    f32 = dt.float32
    AF = mybir.ActivationFunctionType
    OP = mybir.AluOpType
    AX = mybir.AxisListType

    W = meta["W"]
    npc = meta["npc"]
    n_real_pc = meta["n_real_pc"]
    CT = meta["CT"]
    calls = meta["calls"]
    win_calls = {w: [cl for cl in calls if cl[0] == w] for w in range(W)}

    bj = functools.partial(bass_jit, target_bir_lowering=True,
                           sim_require_finite=False,
                           sim_require_nnan=False,
                           dynamic_dma_scratch_size=2**16)

    def load_rep(nc, pool, ap, tag):
        t = pool.tile(list(ap.shape), ap.dtype, tag=tag, name=f"ld_{tag}")
        nc.sync.dma_start(t[:], ap[:, :])
        return t

    def pad_alpha(nc, pad_sb, buf_ap, w):
        if (w + 1) * P > n_real_pc:
            nc.vector.tensor_tensor(out=buf_ap, in0=buf_ap,
                                    in1=pad_sb[:, w:w + 1], op=OP.add)

    def flush(nc, w, tb, dst_tensor, width):
        if (w + 1) % BW != 0 and w != W - 1:
            return
        w0 = (w // BW) * BW
        nb = w - w0 + 1
        dst = dst_tensor[w0 * P:(w0 + nb) * P, :]
        dst_r = dst.rearrange("(j p) f -> p j f", p=P)
        nc.sync.dma_start(dst_r, tb[:, 0:nb, 0:width])

    @bj
    def k_setup(nc, xT, W1, a1s, a1d, padadd):
        T1 = nc.dram_tensor("T1own", [npc, TW], f32, kind="ExternalOutput")
        ad1 = nc.dram_tensor("ad1", [P, W], f32, kind="ExternalOutput")
        with tile.TileContext(nc) as tc:
            with (
                tc.tile_pool(name="const", bufs=1) as constp,
                tc.tile_pool(name="mm", bufs=3) as mmp,
                tc.tile_pool(name="psum", bufs=2, space="PSUM") as psp,
                tc.tile_pool(name="io", bufs=2) as iop,
                tc.tile_pool(name="cmp", bufs=3) as cmpp,
            ):
                W1_sb = load_rep(nc, constp, W1, "W1")
                a1s_sb = load_rep(nc, constp, a1s, "a1s")
                a1d_sb = load_rep(nc, constp, a1d, "a1d")
                pad_sb = load_rep(nc, constp, padadd, "pad")
                ad_t = constp.tile([P, W], f32, tag="adt", name="adt")
                tb = None
                for w in range(W):
                    if w % BW == 0:
                        tb = iop.tile([P, BW, TW], f32, tag="t1b")
                        nc.vector.memset(tb[:, :, FH + 1:TW], 0.0)
                    xt = mmp.tile([FIN, P], f32, tag="xt")
                    nc.sync.dma_start(xt[:], xT[:, w * P:(w + 1) * P])
                    hps = psp.tile([P, FH], f32, tag="hps")
                    nc.tensor.matmul(hps[:], lhsT=xt[:], rhs=W1_sb[:],
                                     start=True, stop=True)
                    hsl = tb[:, w % BW, 0:FH]
                    nc.scalar.copy(hsl, hps[:])
                    scr = cmpp.tile([P, FH], f32, tag="scr")
                    nc.vector.tensor_tensor(out=scr[:], in0=hsl,
                                            in1=a1s_sb[:], op=OP.mult)
                    nc.vector.tensor_reduce(
                        out=tb[:, w % BW, FH:FH + 1], in_=scr[:],
                        axis=AX.X, op=OP.add)
                    scr2 = cmpp.tile([P, FH], f32, tag="scr2")
                    nc.vector.tensor_tensor(out=scr2[:], in0=hsl,
                                            in1=a1d_sb[:], op=OP.mult)
                    nc.vector.tensor_reduce(
                        out=ad_t[:, w:w + 1], in_=scr2[:],
                        axis=AX.X, op=OP.add)
                    pad_alpha(nc, pad_sb, tb[:, w % BW, FH:FH + 1], w)
                    flush(nc, w, tb, T1, TW)
                nc.sync.dma_start(ad1[:, :], ad_t[:])
        return T1, ad1

    def k_layer_body(nc, l, Tq, idxc, ad, padadd, M, aux):
        # aux: l==1 -> (a2s, a2d, b1); l==2 -> (b2, bf)
        if l == 1:
            Tn = nc.dram_tensor("T2own", [npc, TW], f32,
                                kind="ExternalOutput")
            ad2 = nc.dram_tensor("ad2", [P, W], f32, kind="ExternalOutput")
            outs = [Tn, ad2]
        else:
            outf = nc.dram_tensor("outf", [npc, FH], f32,
                                  kind="ExternalOutput")
            outs = [outf]
        with tile.TileContext(nc) as tc:
            with (
                tc.tile_pool(name="const", bufs=1) as constp,
                tc.tile_pool(name="psum", bufs=2, space="PSUM") as psp,
                tc.tile_pool(name="io", bufs=2) as iop,
                tc.tile_pool(name="idxp", bufs=3) as idxp,
                tc.tile_pool(name="gath", bufs=2) as gathp,
                tc.tile_pool(name="cmp", bufs=3) as cmpp,
            ):
                # dma_gather needs a codegen-time DRAM address: stage the
                # (relocatable) input range tables into Internal scratchpad
                Tq_int = []
                for qi in range(NRANGE):
                    ti = nc.dram_tensor(f"Tstage{qi}", [Tq[qi].shape[0], TW],
                                        f32)
                    nc.sync.dma_start(ti[:, :], Tq[qi][:, :])
                    Tq_int.append(ti)
                Tq = Tq_int
                M_sb = load_rep(nc, constp, M, "M")
                ad_sb = load_rep(nc, constp, ad, "ad")
                pad_sb = load_rep(nc, constp, padadd, "pad")
                if l == 1:
                    a2s_sb = load_rep(nc, constp, aux[0], "a2s")
                    a2d_sb = load_rep(nc, constp, aux[1], "a2d")
                    b_sb = load_rep(nc, constp, aux[2], "b1")
                else:
                    b_sb = load_rep(nc, constp, aux[0], "b2")
                    bf_sb = load_rep(nc, constp, aux[1], "bf")
                ident = constp.tile([P, P], f32, tag="ident")
                make_identity(nc, ident[:])
                ad08 = constp.tile([P, W], f32, tag="ad08", name="ad08")
                nc.vector.tensor_scalar_mul(ad08[:], ad_sb[:], 0.8)
                adn = constp.tile([P, W], f32, tag="adn", name="adn") \
                    if l == 1 else None

                ob = None
                for w in range(W):
                    CTw = CT[w]
                    wcalls = win_calls[w]
                    if w % BW == 0:
                        ob = iop.tile([P, BW, TW if l == 1 else FH], f32,
                                      tag="ob")
                        if l == 1:
                            nc.vector.memset(ob[:, :, FH + 1:TW], 0.0)
                    i_lo = wcalls[0][3]
                    i_hi = wcalls[-1][3] + wcalls[-1][2] * 8
                    itr = idxp.tile([P, i_hi - i_lo], dt.int16, tag="itr")
                    nc.sync.dma_start(itr[:], idxc[:, i_lo:i_hi])
                    g = gathp.tile([P, CTw, TW], f32, tag="g")
                    col_pos = 0
                    for (_, q, ncols, ioff) in wcalls:
                        nc.gpsimd.dma_gather(
                            out_ap=g[:, col_pos:col_pos + ncols, :],
                            in_ap=Tq[q][:, :],
                            idxs_ap=itr[:, ioff - i_lo:ioff - i_lo
                                        + ncols * 8],
                            num_idxs=ncols * P, num_idxs_reg=ncols * P,
                            elem_size=TW, single_packet=False)
                        col_pos += ncols
                    as_view = g[:, :, FH:FH + 1].rearrange("p d o -> p (d o)")
                    r8 = cmpp.tile([P, CTw], f32, tag="r8")
                    nc.scalar.activation(out=r8[:], in_=as_view, func=AF.Relu,
                                         bias=ad08[:, w:w + 1], scale=0.8)
                    t02 = cmpp.tile([P, CTw], f32, tag="t02")
                    nc.vector.tensor_scalar(
                        out=t02[:], in0=as_view, scalar1=ad_sb[:, w:w + 1],
                        scalar2=0.2, op0=OP.add, op1=OP.mult)
                    e_t = cmpp.tile([P, CTw], f32, tag="e")
                    nc.vector.tensor_tensor(out=e_t[:], in0=r8[:],
                                            in1=t02[:], op=OP.add)
                    wx = cmpp.tile([P, CTw], f32, tag="wx")
                    den = cmpp.tile([P, 1], f32, tag="den")
                    nc.scalar.activation(out=wx[:], in_=e_t[:], func=AF.Exp,
                                         accum_out=den[:])
                    wt = cmpp.tile([P, CTw, FH], f32, tag="wt")
                    nc.vector.tensor_tensor(
                        out=wt[:], in0=g[:, :, 0:FH],
                        in1=wx[:, :, None].to_broadcast([P, CTw, FH]),
                        op=OP.mult)
                    num = cmpp.tile([P, FH], f32, tag="num")
                    nc.vector.tensor_reduce(
                        out=num[:], in_=wt[:].rearrange("p d f -> p f d"),
                        axis=AX.X, op=OP.add)
                    den2 = cmpp.tile([P, 1], f32, tag="den2")
                    nc.vector.tensor_scalar_max(den2[:], den[:], 1e-30)
                    rden = cmpp.tile([P, 1], f32, tag="rden")
                    nc.vector.reciprocal(rden[:], den2[:])
                    gs = cmpp.tile([P, FH], f32, tag="gs")
                    nc.scalar.activation(out=gs[:], in_=num[:], func=AF.Copy,
                                         scale=rden[:, 0:1])
                    g2 = cmpp.tile([P, FH], f32, tag="g2")
                    nc.vector.tensor_tensor(out=g2[:], in0=gs[:], in1=b_sb[:],
                                            op=OP.add)
                    hact = cmpp.tile([P, FH], f32, tag="hact")
                    nc.scalar.activation(out=hact[:], in_=g2[:], func=AF.Relu)
                    tp = psp.tile([FH, P], f32, tag="tp")
                    nc.tensor.transpose(tp[:], hact[:], ident[:])
                    hT = cmpp.tile([FH, P], f32, tag="hT")
                    nc.scalar.copy(hT[:], tp[:])
                    ps2 = psp.tile([P, FH], f32, tag="ps2")
                    nc.tensor.matmul(ps2[:], lhsT=hT[:], rhs=M_sb[:],
                                     start=True, stop=True)
                    if l == 1:
                        h2sl = ob[:, w % BW, 0:FH]
                        nc.scalar.copy(h2sl, ps2[:])
                        scr3 = cmpp.tile([P, FH], f32, tag="scr3")
                        nc.vector.tensor_tensor(out=scr3[:], in0=h2sl,
                                                in1=a2s_sb[:], op=OP.mult)
                        nc.vector.tensor_reduce(
                            out=ob[:, w % BW, FH:FH + 1], in_=scr3[:],
                            axis=AX.X, op=OP.add)
                        scr4 = cmpp.tile([P, FH], f32, tag="scr4")
                        nc.vector.tensor_tensor(out=scr4[:], in0=h2sl,
                                                in1=a2d_sb[:], op=OP.mult)
                        nc.vector.tensor_reduce(
                            out=adn[:, w:w + 1], in_=scr4[:],
                            axis=AX.X, op=OP.add)
                        pad_alpha(nc, pad_sb, ob[:, w % BW, FH:FH + 1], w)
                        flush(nc, w, ob, outs[0], TW)
                    else:
                        fsl = ob[:, w % BW, 0:FH]
                        nc.vector.tensor_tensor(out=fsl, in0=ps2[:],
                                                in1=bf_sb[:], op=OP.add)
                        flush(nc, w, ob, outs[0], FH)
                if l == 1:
                    nc.sync.dma_start(outs[1][:, :], adn[:])
        return tuple(outs)

    @bj
    def k_layer1(nc, Tq0, Tq1, Tq2, Tq3, idxc, ad, padadd, M, a2s, a2d, b1):
        return k_layer_body(nc, 1, [Tq0, Tq1, Tq2, Tq3], idxc, ad, padadd,
                            M, (a2s, a2d, b1))

    @bj
    def k_layer2(nc, Tq0, Tq1, Tq2, Tq3, idxc, ad, padadd, M, b2, bf):
        return k_layer_body(nc, 2, [Tq0, Tq1, Tq2, Tq3], idxc, ad, padadd,
                            M, (b2, bf))

    return k_setup, k_layer1, k_layer2


_RUN_CACHE = {}


def _get_runner(meta):
    key = (meta["W"], meta["npc"], meta["n_real_pc"], meta["IDXW"],
           tuple(meta["CT"]), tuple(cl[1:] for cl in meta["calls"]))
    if key in _RUN_CACHE:
        return _RUN_CACHE[key]

    import jax
    import jax.numpy as jnp
    from jax.experimental.shard_map import shard_map
    from jax.sharding import Mesh, PartitionSpec

    k_setup, k_layer1, k_layer2 = _build_fns(meta)
    QS, RT = meta["QS"], meta["RT"]

    mesh = Mesh(np.array(jax.devices()[:CORES]), ("x",))
    Px = PartitionSpec("x")

    def ranges(T):
        # quarter q's table = concat over cores of their quarter-shard rows;
        # realized as slice+reshape of the all_gather (no XLA transpose)
        allg = jax.lax.all_gather(T, "x", axis=0, tiled=False)  # [8, npc, TW]
        dr = jnp.zeros((1, TW), jnp.float32).at[0, FH].set(
            jnp.float32(NEG_BIG))
        return [jnp.concatenate(
            [allg[:, q * QS:(q + 1) * QS, :].reshape(CORES * QS, TW), dr],
            axis=0) for q in range(NRANGE)]

    def body(xT, idxc, padadd, W1, W2, Wf, a1s, a1d, a2s, a2d, b1, b2, bf):
        sq = lambda a: a.reshape(a.shape[1:])
        T1, ad1 = k_setup(sq(xT), sq(W1), sq(a1s), sq(a1d), sq(padadd))
        Tq = ranges(T1)
        T2, ad2 = k_layer1(Tq[0], Tq[1], Tq[2], Tq[3], sq(idxc), ad1,
                           sq(padadd), sq(W2), sq(a2s), sq(a2d), sq(b1))
        Tq2 = ranges(T2)
        (o,) = k_layer2(Tq2[0], Tq2[1], Tq2[2], Tq2[3], sq(idxc), ad2,
                        sq(padadd), sq(Wf), sq(b2), sq(bf))
        return o[None]

    run = jax.jit(shard_map(body, mesh=mesh, in_specs=(Px,) * 13,
                            out_specs=Px, check_rep=False))
    _RUN_CACHE[key] = run
    return run


def _make_args(meta, x, W1, a1s, a1d, b1, W2, a2s, a2d, b2, Wf, bf):
    order = meta["order"]
    npc, n_real_pc = meta["npc"], meta["n_real_pc"]

    def repmat(v):
        t = np.tile(np.asarray(v, np.float32)[None, :], (P, 1))
        return np.tile(t[None], (CORES, 1, 1))

    def repfull(m):
        return np.tile(np.ascontiguousarray(np.asarray(m, np.float32))[None],
                       (CORES, 1, 1))

    slot_ids = np.arange(npc).reshape(meta["W"], P).T
    padadd = np.where(slot_ids >= n_real_pc, np.float32(NEG_BIG),
                      np.float32(0.0)).astype(np.float32)
    padadd = np.tile(padadd[None], (CORES, 1, 1))

    xT = np.zeros((CORES, P, npc), np.float32)
    for c in range(CORES):
        nodes_c = order[c::CORES]
        xT[c, :, :n_real_pc] = np.asarray(x, np.float32)[nodes_c].T
    idxc = np.stack([meta["idx"][c] for c in range(CORES)])

    return (xT, idxc, padadd, repfull(W1), repfull(W2), repfull(Wf),
            repmat(a1s), repmat(a1d), repmat(a2s), repmat(a2d),
            repmat(b1), repmat(b2), repmat(bf))


def _assemble_output(meta, big, n_nodes):
    rank = meta["rank"]
    return np.ascontiguousarray(
        big[rank % CORES, rank // CORES, :]).astype(np.float32)


def kernel(x, edge_index, W1, a1s, a1d, b1, W2, a2s, a2d, b2, Wf, bf):
    x = np.asarray(x, np.float32)
    n_nodes = x.shape[0]
    meta = _make_meta(edge_index, n_nodes)
    run = _get_runner(meta)
    args = _make_args(meta, x, W1, a1s, a1d, b1, W2, a2s, a2d, b2, Wf, bf)
    big = np.asarray(run(*args))
    return _assemble_output(meta, big, n_nodes)
